# revision 23
# baseline (speedup 1.0000x reference)
"""Trainium2 Bass kernel for nn_NodeModel (GNN message passing + node MLP).

Reference math:
    col = edge_index[1]
    geo_sum   = segment_sum(edge_attr * x[col], col, N)   # gather idx == scatter idx!
    geo_denom = segment_sum(edge_attr, col, N)
    geo_agg   = x + geo_sum / geo_denom   (NaN -> 0)
    out = relu(relu(geo_agg @ W1 + b1) @ W2 + b2)

Because the gather index equals the scatter index, geo_sum[n] == x[n] * geo_denom[n]
mathematically, so geo_agg == 2*x where geo_denom != 0, and 0 (the NaN path)
elsewhere. The kernel therefore needs only the scalar segment-sum (denominator)
for the mask, plus the dense MLP.

Distribution: nodes are partitioned across the 8 cores (6250 each). The host
routes each edge's attr into a per-node slot matrix attrp[slot, node] (pure data
layout - all summation arithmetic runs on device). No collectives needed.

Device layout (per core): tensors are packed [128, 3125] - partitions 0:64 hold
features/slots of nodes 0:3125 ("half 0"), partitions 64:128 of nodes 3125:6250
("half 1") - so every DMA and compute op uses all 128 partitions. Weights are
block-diagonal [[W, 0], [0, W]] so one matmul processes both halves. Per
512-column chunk:
    denom  = blockdiag(ones)^T @ attrp_chunk          (PE, K=128, fp32r)
    g      = (denom > 0) * xT_chunk                   (DVE scalar_tensor_tensor)
    h_q    = relu(blockdiag(W1_q)^T @ g + b1_q)       (PE fp32r; relu on ACT/DVE)
    o      = sum_q blockdiag(W2_q)^T @ h_q            (PE accum, fp32r)
    outT_chunk = relu(o + b2)                         (DVE)
W1 is pre-scaled by 2 on the host (folds geo_agg = 2x); the host unpacks the
[128, 3125] outputs back to [50000, 64].
"""

import numpy as np

N = 50000
D_IN = 64
D_HID = 256
D_OUT = 64
NCORES = 8
NPC = N // NCORES    # 6250 nodes per core
HALF = NPC // 2      # 3125 nodes per packed half
HALFP = 3136         # padded half width (fp32r matmul needs even chunk sizes)
SLOTS = 64           # padded edge-slot count (max in-degree supported)
CHUNK = 448
WPACK_COLS = 1157    # packed consts: ones(128) + 4x W1q(128) + 4x W2q(128) + 4x b1 + b2

_program_cache = {}


def build_program(repeats: int = 1, loop_n: int | None = None, use_f32r: bool = True):
    """Build the per-core Bass/Tile program. Same program runs SPMD on all 8 cores."""
    import concourse.bass as bass
    import concourse.tile as tile
    from concourse import bacc, mybir

    f32 = mybir.dt.float32
    f32r = mybir.dt.float32r
    bf16 = mybir.dt.bfloat16
    Alu = mybir.AluOpType
    Act = mybir.ActivationFunctionType

    # fp32r (TF32) matmul operands: the BIR verifier requires every matmul
    # operand's producer to emit fp32r-rounded values, so the tiles feeding
    # matmuls are typed float32r end-to-end (DMA for weights/attrp, DVE/ACT
    # rounding outputs for g/h). Biases stay fp32 in a separate tensor.
    mdt = f32r if use_f32r else f32

    # Bacc (not plain Bass): Bacc.compile() runs generate_event_semaphores,
    # which legalizes multi-wait instructions for walrus (1 wait/inst on TRN2).
    nc = bacc.Bacc("TRN2", target_bir_lowering=False, debug=False, num_devices=NCORES)

    xT_d = nc.dram_tensor("xT", [128, HALFP], f32, kind="ExternalInput").ap()
    attrp_d = nc.dram_tensor("attrp", [128, HALFP], bf16, kind="ExternalInput").ap()
    # wts: host-prepacked block-diag weights [128, 1152]:
    #   [:, 0:128]        block-diag ones
    #   [:, 128+q*128...] block-diag 2*W1 quarter q (q=0..3)
    #   [:, 640+q*128...] block-diag W2 quarter q
    wts_d = nc.dram_tensor("wts", [128, 1152], mdt, kind="ExternalInput").ap()
    onesb_d = nc.dram_tensor("onesb", [128, 128], bf16, kind="ExternalInput").ap()
    # bias: [:, 0:4] b1 quarters duplicated per half; [:, 4] b2 duplicated
    bias_d = nc.dram_tensor("bias", [128, 5], f32, kind="ExternalInput").ap()
    outT_d = nc.dram_tensor("outT", [128, HALFP], f32, kind="ExternalOutput").ap()

    n_chunks = (HALFP + CHUNK - 1) // CHUNK  # 7 x 448 = 3136 exactly
    NQ = 4  # quarters of D_HID

    with tile.TileContext(nc) as tc:
        with (
            tc.tile_pool(name="const", bufs=1) as cpool,
            tc.tile_pool(name="xin", bufs=2) as xpool,
            tc.tile_pool(name="ain", bufs=2) as apool,
            tc.tile_pool(name="g", bufs=3) as gpool,
            tc.tile_pool(name="h", bufs=6) as hpool,
            tc.tile_pool(name="oacc", bufs=2) as opool,
            tc.tile_pool(name="pd", bufs=2, space=bass.MemorySpace.PSUM) as pd,
            tc.tile_pool(name="ph", bufs=4, space=bass.MemorySpace.PSUM) as ph,
            tc.tile_pool(name="po", bufs=2, space=bass.MemorySpace.PSUM) as po,
        ):
            # --- constants (two DMAs, outside any timing loop) ---
            wts = cpool.tile([128, 1152], mdt, tag="wts")
            nc.sync.dma_start(wts[:], wts_d[:])
            bias = cpool.tile([128, 5], f32, tag="bias")
            nc.sync.dma_start(bias[:], bias_d[:])
            onesbd = cpool.tile([128, 128], bf16, tag="onesb")
            nc.sync.dma_start(onesbd[:], onesb_d[:])
            w1q = [wts[:, 128 + q * 128:256 + q * 128] for q in range(NQ)]
            w2q = [wts[:, 640 + q * 128:768 + q * 128] for q in range(NQ)]
            b1q = [bias[:, q:q + 1] for q in range(NQ)]
            b2p = bias[:, 4:5]

            def emit_body():
                # stream inputs (inside the body so timing includes the loads)
                xT_sb = xpool.tile([128, HALFP], f32, tag="xT")
                attrp_sb = apool.tile([128, HALFP], bf16, tag="attrp")
                bounds = [0, 448, 1344, 2240, HALFP]
                for s in range(len(bounds) - 1):
                    lo, hi = bounds[s], bounds[s + 1]
                    nc.sync.dma_start(xT_sb[:, lo:hi], xT_d[:, lo:hi])
                    nc.sync.dma_start(attrp_sb[:, lo:hi], attrp_d[:, lo:hi])
                outT_sb = opool.tile([128, HALFP], f32, tag="outT")

                # software pipeline: denom+mask for chunk j+1 are emitted
                # before the relus of chunk j, so the DVE's stt(j+1) is not
                # queued behind relu(j) and PE's mm1(j+1) starts earlier.
                d_tiles = {}
                g_tiles = {}

                def emit_mask(j):
                    n0 = j * CHUNK
                    n1 = min(HALFP, n0 + CHUNK)
                    w = n1 - n0
                    d_ps = pd.tile([128, w], f32, tag="d")
                    nc.tensor.matmul(
                        d_ps[:], onesbd[:], attrp_sb[:, n0:n1],
                        start=True, stop=True,
                    )
                    g_sb = gpool.tile([128, w], mdt, tag="g")
                    nc.vector.scalar_tensor_tensor(
                        g_sb[:], d_ps[:], 0.0, xT_sb[:, n0:n1],
                        op0=Alu.is_gt, op1=Alu.mult,
                    )
                    g_tiles[j] = g_sb

                emit_mask(0)
                for j in range(n_chunks):
                    n0 = j * CHUNK
                    n1 = min(HALFP, n0 + CHUNK)
                    w = n1 - n0
                    if j + 1 < n_chunks:
                        emit_mask(j + 1)
                    g_sb = g_tiles.pop(j)

                    # h_q = relu(g @ 2*W1 + b1) quarters; relu split ACT/DVE
                    h_tiles = []
                    for q in range(NQ):
                        h_ps = ph.tile([128, w], f32, tag="h")
                        nc.tensor.matmul(
                            h_ps[:], w1q[q][:], g_sb[:],
                            start=True, stop=True,
                        )
                        h_sb = hpool.tile([128, w], mdt, tag="h")
                        if q % 2 == 0:
                            nc.scalar.activation(
                                h_sb[:], h_ps[:], Act.Relu, bias=b1q[q][:], scale=1.0
                            )
                        else:
                            nc.vector.tensor_scalar(
                                h_sb[:], h_ps[:], b1q[q][:], 0.0,
                                op0=Alu.add, op1=Alu.max,
                            )
                        h_tiles.append(h_sb)

                    # out = relu(h @ W2 + b2), accumulated over the 4 K quarters
                    o_ps = po.tile([128, w], f32, tag="o")
                    for q in range(NQ):
                        nc.tensor.matmul(
                            o_ps[:], w2q[q][:], h_tiles[q][:],
                            start=(q == 0), stop=(q == NQ - 1),
                        )
                    if j % 2 == 0:
                        nc.scalar.activation(
                            outT_sb[:, n0:n1], o_ps[:], Act.Relu, bias=b2p[:], scale=1.0
                        )
                    else:
                        nc.vector.tensor_scalar(
                            outT_sb[:, n0:n1], o_ps[:], b2p[:], 0.0,
                            op0=Alu.add, op1=Alu.max,
                        )
                    nc.sync.dma_start(outT_d[:, n0:n1], outT_sb[:, n0:n1])

            if loop_n is not None:
                with tc.For_i(0, loop_n, 1):
                    emit_body()
            else:
                for _ in range(repeats):
                    emit_body()

    nc.compile()
    return nc


def shard_inputs(x, edge_index, edge_attr, W1, b1, W2, b2):
    """Host-side routing/layout: build per-core input maps (pure data movement;
    all arithmetic of the reference computation happens on device)."""
    x = np.asarray(x, dtype=np.float32)
    ei = np.asarray(edge_index)
    ea = np.asarray(edge_attr, dtype=np.float32).reshape(-1)
    W1 = np.asarray(W1, dtype=np.float32)
    b1 = np.asarray(b1, dtype=np.float32)
    W2 = np.asarray(W2, dtype=np.float32)
    b2 = np.asarray(b2, dtype=np.float32)

    col = ei[1].astype(np.int64)
    E = col.shape[0]
    order = np.argsort(col, kind="stable")
    col_s = col[order]
    ea_s = ea[order]
    counts = np.bincount(col_s, minlength=N)
    maxdeg = int(counts.max())
    assert maxdeg <= SLOTS, f"in-degree {maxdeg} exceeds SLOTS={SLOTS}"
    starts = np.zeros(N, dtype=np.int64)
    starts[1:] = np.cumsum(counts)[:-1]
    ranks = np.arange(E, dtype=np.int64) - starts[col_s]

    attrp = np.zeros((SLOTS, N), dtype=np.float32)
    attrp[ranks, col_s] = ea_s

    xT = np.ascontiguousarray(x.T)  # [64, N]
    w1s = 2.0 * W1  # fold geo_agg = 2*x into W1

    # packed consts: block-diag [[A, 0], [0, A]] weights + per-half biases
    import ml_dtypes
    onesb = np.zeros((128, 128), dtype=ml_dtypes.bfloat16)
    onesb[0:64, 0:64] = 1.0
    onesb[64:128, 64:128] = 1.0
    wts = np.zeros((128, 1152), dtype=np.float32)
    for q in range(4):
        blk = wts[:, 128 + q * 128:256 + q * 128]
        blk[0:64, 0:64] = w1s[:, q * 64:(q + 1) * 64]
        blk[64:128, 64:128] = w1s[:, q * 64:(q + 1) * 64]
        blk = wts[:, 640 + q * 128:768 + q * 128]
        blk[0:64, 0:64] = W2[q * 64:(q + 1) * 64, :]
        blk[64:128, 64:128] = W2[q * 64:(q + 1) * 64, :]
    bias = np.zeros((128, 5), dtype=np.float32)
    for q in range(4):
        bias[0:64, q] = b1[q * 64:(q + 1) * 64]
        bias[64:128, q] = b1[q * 64:(q + 1) * 64]
    bias[0:64, 4] = b2
    bias[64:128, 4] = b2

    def pack(a, c):  # [64, NPC] slice of core c -> [128, HALFP] (zero-padded)
        s = a[:, c * NPC:(c + 1) * NPC]
        out = np.zeros((128, HALFP), dtype=np.float32)
        out[:64, :HALF] = s[:, :HALF]
        out[64:, :HALF] = s[:, HALF:]
        return out

    in_maps = []
    for c in range(NCORES):
        in_maps.append(
            {
                "xT": pack(xT, c),
                "attrp": pack(attrp, c).astype(ml_dtypes.bfloat16),
                "wts": wts,
                "onesb": onesb,
                "bias": bias,
            }
        )
    return in_maps


def kernel(x, edge_index, edge_attr, u, batch, W1, b1, W2, b2):
    from concourse.bass_utils import run_bass_kernel_spmd

    in_maps = shard_inputs(x, edge_index, edge_attr, W1, b1, W2, b2)

    if "prog" not in _program_cache:
        _program_cache["prog"] = build_program()
    nc = _program_cache["prog"]

    res = run_bass_kernel_spmd(nc, in_maps, list(range(NCORES))).results
    # unpack [128, HALF] per core -> [64, NPC] -> [N, 64]
    outT = np.concatenate(
        [
            np.concatenate(
                [res[c]["outT"][:64, :HALF], res[c]["outT"][64:, :HALF]], axis=1
            )
            for c in range(NCORES)
        ],
        axis=1,
    )
    return np.ascontiguousarray(outT.T)
